# revision 26
# baseline (speedup 1.0000x reference)
"""Trainium2 Bass kernel for nn_EpigeneticNormalization.

Math (per token, D=1024, F=512, NS=3):
  h      = silu(x @ pred_w1 + pred_b1)
  logits = h @ pred_w2 + pred_b2
  states = softmax(logits);  sid = argmax(logits)
  xn     = LN(x) * norm_gamma[sid] + norm_beta[sid]
  t      = concat(xn, emb[sid])
  y      = silu(t @ tr_w1 + tr_b1) @ tr_w2 + tr_b2
  out    = xn + LN(y) * tr_gamma + tr_beta,  states

Device strategy (8 cores, token-parallel, 4096 tokens/core):
  - Activations in "transposed" layout [feature, token] for matmuls
    (contract dim on partitions), natural [token, feature] for LN.
  - x^T is host-pretransposed and fp16-hi/lo split; pred1 runs as a
    3-pass fp16 matmul (hi*hi + hi*lo + lo*hi) which is bit-accurate
    enough to keep argmax stable (min top-2 logit gap ~5e-6; measured
    fp16x3 logit error <1e-6).  pred2 stays fp32.
  - tr path in fp16 (full PE rate).  emb[sid] @ tr_w1[D:] is folded on
    host into a [3,1024] matrix; the gather becomes a K=3 one-hot
    matmul accumulated into the tr1 psum.
  - ACT uses only the silu table set (silu/tanh/identity/copy):
    softmax's exp goes through tanh, LN's rsqrt is a DVE Newton
    iteration on a bitcast seed.  Zero ACT table switches in steady
    state.
  - PE transposes (xn^T, z back to natural) are interleaved between
    matmul groups to keep HAM from re-throttling the PE clock.
"""
import json
import sys

sys.path.insert(0, "/opt/trn_rl_repo")

import numpy as np

import concourse.bass as bass
import concourse.tile as tile
from concourse import mybir
from concourse.bass_utils import run_bass_kernel_spmd
from concourse.masks import make_identity

F32 = mybir.dt.float32
F16 = mybir.dt.float16
I32 = mybir.dt.int32
AF = mybir.ActivationFunctionType
ALU = mybir.AluOpType

NCORES = 8
B, S, D, NS = 4, 8192, 1024, 3
F = D // 2
T = B * S                 # 32768 tokens total
TPC = T // NCORES         # 4096 tokens per core
TT = 512                  # tokens per macro tile
NMACRO = TPC // TT        # 8
NST = TT // 128           # 4 sub-tiles of 128 tokens
DC = D // 128             # 8 feature chunks
FC = F // 128             # 4 pred-hidden chunks
EPS = 1e-5
MAGIC = 0x5F3759DF

_WAIT_LIMIT = 1


def _fix_excess_waits(nc, limit=_WAIT_LIMIT):
    """This walrus build rejects >1 sync-wait per instruction; split the
    excess onto same-engine Drain instructions inserted just before."""
    raw = nc.to_json_bytes()
    d = json.loads(raw)
    n = [0]

    def mk(engine, waits, debug):
        n[0] += 1
        return {"opcode": "Drain", "engine": engine, "ins": [], "outs": [],
                "is_reset_sema": False, "name": f"I-waitfix-{n[0]}",
                "sync_info": {"on_update": [], "on_wait": waits},
                "debug": debug}

    for f in d["functions"]:
        for b in f["blocks"]:
            out = []
            for inst in b["instructions"]:
                si = inst.get("sync_info") or {}
                waits = si.get("on_wait") or []
                if len(waits) > limit:
                    excess = waits[: len(waits) - limit]
                    si["on_wait"] = waits[len(waits) - limit:]
                    inst["sync_info"] = si
                    for i in range(0, len(excess), limit):
                        out.append(mk(inst["engine"], excess[i:i + limit],
                                      inst.get("debug")))
                out.append(inst)
            b["instructions"] = out
    blob = json.dumps(d).encode()
    nc.to_json_bytes = lambda: blob
    return nc


def _rsqrt_newton(nc, P, x, n_iter=3):
    """out = 1/sqrt(x) elementwise on DVE (no ACT table use).

    x: [128, W] fp32 sbuf AP.  Returns a [128, W] fp32 tile.
    """
    W = x.shape[1]
    xh = P.tile([128, W], F32, tag="nwxh")
    nc.vector.tensor_scalar_mul(out=xh, in0=x, scalar1=0.5)
    ibits = P.tile([128, W], I32, tag="nwi")
    nc.vector.tensor_scalar(out=ibits, in0=x.bitcast(I32), scalar1=1,
                            scalar2=None, op0=ALU.arith_shift_right)
    y = P.tile([128, W], F32, tag="nwy")
    nc.vector.tensor_scalar(out=y.bitcast(I32), in0=ibits, scalar1=-1,
                            scalar2=MAGIC, op0=ALU.mult, op1=ALU.add)
    t1 = P.tile([128, W], F32, tag="nwt1")
    t2 = P.tile([128, W], F32, tag="nwt2")
    for _ in range(n_iter):
        nc.vector.tensor_mul(out=t1, in0=y, in1=y)
        nc.vector.tensor_mul(out=t2, in0=t1, in1=xh)
        nc.vector.tensor_scalar(out=t2, in0=t2, scalar1=-1.0, scalar2=1.5,
                                op0=ALU.mult, op1=ALU.add)
        nc.vector.tensor_mul(out=y, in0=y, in1=t2)
    return y


def build_nc(norm_affine: bool, tr_affine: bool):
    nc = bass.Bass()

    x_e = nc.declare_dram_parameter("x", [TPC, D], F32, isOutput=False)
    xth_e = nc.declare_dram_parameter("xth", [D, TPC], F16, isOutput=False)
    xtl_e = nc.declare_dram_parameter("xtl", [D, TPC], F16, isOutput=False)
    w1h_e = nc.declare_dram_parameter("pw1h", [D, F], F16, isOutput=False)
    w1l_e = nc.declare_dram_parameter("pw1l", [D, F], F16, isOutput=False)
    b1_e = nc.declare_dram_parameter("pb1", [128, FC], F32, isOutput=False)
    w2_e = nc.declare_dram_parameter("pw2", [F, NS], F32, isOutput=False)
    b2_e = nc.declare_dram_parameter("pb2", [NS, 1], F32, isOutput=False)
    tw1_e = nc.declare_dram_parameter("tw1", [D, D], F16, isOutput=False)
    tb1_e = nc.declare_dram_parameter("tb1", [128, DC], F32, isOutput=False)
    tw2_e = nc.declare_dram_parameter("tw2", [D, D], F16, isOutput=False)
    tb2_e = nc.declare_dram_parameter("tb2", [128, DC], F32, isOutput=False)
    emtr_e = nc.declare_dram_parameter("emtr", [NS, D], F16, isOutput=False)
    if norm_affine:
        ng_e = nc.declare_dram_parameter("ngam", [NS, D], F32, isOutput=False)
        nb_e = nc.declare_dram_parameter("nbet", [NS, D], F32, isOutput=False)
    if tr_affine:
        tg_e = nc.declare_dram_parameter("tgam", [1, D], F32, isOutput=False)
        tb_e = nc.declare_dram_parameter("tbet", [1, D], F32, isOutput=False)
    out_e = nc.declare_dram_parameter("out", [TPC, D], F32, isOutput=True)
    st_e = nc.declare_dram_parameter("states", [TPC, NS], F32, isOutput=True)

    from contextlib import ExitStack
    BUFS = {"xnat": 4, "xn": 10, "hT": 6, "xth": 10, "xtl": 10,
            "xnT": 17, "y1T": 8, "zT": 8, "znat": 4, "outn": 2, "lT": 2,
            "ohT": 3, "ohT32": 2, "tiny": 12}
    with tile.TileContext(nc) as tc, ExitStack() as ctx:
        pools = {t: ctx.enter_context(tc.tile_pool(name=t, bufs=n))
                 for t, n in BUFS.items()}

        class _P:
            def tile(self, shape, dt, tag=None, name=None):
                return pools[tag if tag in pools else "tiny"].tile(
                    shape, dt, tag=tag, name=name or tag)
        P = _P()
        cons = ctx.enter_context(tc.tile_pool(name="cons", bufs=1))
        ps_tp = ctx.enter_context(tc.tile_pool(
            name="ps_tp", bufs=2, space="PSUM"))
        ps_z = ctx.enter_context(tc.tile_pool(
            name="ps_z", bufs=2, space="PSUM"))
        ps_mm = ctx.enter_context(tc.tile_pool(
            name="ps_mm", bufs=3 if norm_affine else 4, space="PSUM"))
        if norm_affine:
            ps_aff = ctx.enter_context(tc.tile_pool(
                name="ps_aff", bufs=1, space="PSUM"))

        # ---- constants / weights (loaded once) ----
        ident = cons.tile([128, 128], F32)
        make_identity(nc, ident)
        ident16 = cons.tile([128, 128], F16)
        make_identity(nc, ident16)

        w1h = cons.tile([128, DC, F], F16)
        w1l = cons.tile([128, DC, F], F16)
        for c in range(DC):
            nc.gpsimd.dma_start(out=w1h[:, c, :], in_=w1h_e[c * 128:(c + 1) * 128, :])
            nc.gpsimd.dma_start(out=w1l[:, c, :], in_=w1l_e[c * 128:(c + 1) * 128, :])
        w2 = cons.tile([128, FC, NS], F32)
        for c in range(FC):
            nc.gpsimd.dma_start(out=w2[:, c, :], in_=w2_e[c * 128:(c + 1) * 128, :])
        tw1 = cons.tile([128, DC, D], F16)
        tw2 = cons.tile([128, DC, D], F16)
        for c in range(DC):
            nc.gpsimd.dma_start(out=tw1[:, c, :], in_=tw1_e[c * 128:(c + 1) * 128, :])
            nc.gpsimd.dma_start(out=tw2[:, c, :], in_=tw2_e[c * 128:(c + 1) * 128, :])
        emtr = cons.tile([NS, D], F16)
        nc.gpsimd.dma_start(out=emtr, in_=emtr_e[:])
        b1 = cons.tile([128, FC], F32)
        nc.gpsimd.dma_start(out=b1, in_=b1_e[:])
        b2 = cons.tile([NS, 1], F32)
        nc.gpsimd.dma_start(out=b2, in_=b2_e[:])
        tb1 = cons.tile([128, DC], F32)
        nc.gpsimd.dma_start(out=tb1, in_=tb1_e[:])
        tb2 = cons.tile([128, DC], F32)
        nc.gpsimd.dma_start(out=tb2, in_=tb2_e[:])
        if norm_affine:
            ngam = cons.tile([NS, D], F32)
            nc.gpsimd.dma_start(out=ngam, in_=ng_e[:])
            nbet = cons.tile([NS, D], F32)
            nc.gpsimd.dma_start(out=nbet, in_=nb_e[:])
        if tr_affine:
            tgam = cons.tile([128, D], F32)
            tbet = cons.tile([128, D], F32)
            bg = tg_e[:]
            bb = tb_e[:]
            nc.gpsimd.dma_start(out=tgam, in_=bass.AP(
                tensor=bg.tensor, offset=bg.offset, ap=[[0, 128]] + bg.ap[1:]))
            nc.gpsimd.dma_start(out=tbet, in_=bass.AP(
                tensor=bb.tensor, offset=bb.offset, ap=[[0, 128]] + bb.ap[1:]))

        # ---- main loop over macro tiles (LN-x stage pipelined 1 ahead) ----
        def stage_a(mi):
            """DMA x/x^T for macro mi, compute LN(x) stats + xn."""
            t0 = mi * TT
            xnat = []
            for st in range(NST):
                xt = P.tile([128, D], F32, tag="xnat")
                nc.sync.dma_start(out=xt, in_=x_e[t0 + st * 128: t0 + (st + 1) * 128, :])
                xnat.append(xt)
            xth, xtl = [], []
            for fc in range(DC):
                th = P.tile([128, TT], F16, tag="xth")
                nc.sync.dma_start(out=th,
                                  in_=xth_e[fc * 128:(fc + 1) * 128, t0:t0 + TT])
                xth.append(th)
                tl = P.tile([128, TT], F16, tag="xtl")
                nc.sync.dma_start(out=tl,
                                  in_=xtl_e[fc * 128:(fc + 1) * 128, t0:t0 + TT])
                xtl.append(tl)
            mvall = P.tile([128, NST, 2], F32, tag="mvall")
            for st in range(NST):
                stats = P.tile([128, 2, 6], F32, tag="bnst")
                xr = xnat[st].rearrange("p (a b) -> p a b", a=2)
                nc.vector.bn_stats(out=stats[:, 0, :], in_=xr[:, 0, :])
                nc.vector.bn_stats(out=stats[:, 1, :], in_=xr[:, 1, :])
                nc.vector.bn_aggr(out=mvall[:, st, :], in_=stats)
            vv = P.tile([128, NST], F32, tag="vv")
            nc.vector.tensor_scalar_add(out=vv, in0=mvall[:, :, 1], scalar1=EPS)
            rr = _rsqrt_newton(nc, P, vv)
            xn = []
            for st in range(NST):
                xnt = P.tile([128, D], F16, tag="xn")
                nc.vector.tensor_scalar(out=xnt, in0=xnat[st],
                                        scalar1=mvall[:, st, 0:1],
                                        scalar2=rr[:, st:st + 1],
                                        op0=ALU.subtract, op1=ALU.mult)
                xn.append(xnt)
            xnT = []
            if not norm_affine:
                for fc in range(DC):
                    pt = ps_tp.tile([128, NST, 128], F16, tag="tp", name="tp")
                    for st in range(NST):
                        nc.tensor.matmul(pt[:, st, :],
                                         xn[st][:, fc * 128:(fc + 1) * 128],
                                         ident16, is_transpose=True,
                                         skip_group_check=True)
                    xt = P.tile([128, TT], F16, tag="xnT")
                    nc.scalar.activation(out=xt, in_=pt, func=AF.Copy)
                    xnT.append(xt)
            return xth, xtl, xn, xnT

        staged = stage_a(0)
        for mi in range(NMACRO):
            t0 = mi * TT
            xth, xtl, xn, xnT = staged
            if mi + 1 < NMACRO:
                staged = stage_a(mi + 1)

            # pred1 (fp16 x3 passes)
            hT = []
            for mo in range(FC):
                ps = ps_mm.tile([128, TT], F32, tag="mm", name="mm")
                for k in range(DC):
                    nc.tensor.matmul(ps, w1h[:, k, mo * 128:(mo + 1) * 128],
                                     xth[k], start=(k == 0), stop=False)
                for k in range(DC):
                    nc.tensor.matmul(ps, w1h[:, k, mo * 128:(mo + 1) * 128],
                                     xtl[k], start=False, stop=False)
                for k in range(DC):
                    nc.tensor.matmul(ps, w1l[:, k, mo * 128:(mo + 1) * 128],
                                     xth[k], start=False,
                                     stop=(k == DC - 1))
                h = P.tile([128, TT], F32, tag="hT")
                nc.scalar.activation(out=h, in_=ps, func=AF.Silu,
                                     bias=b1[:, mo:mo + 1], scale=1.0)
                hT.append(h)

            # pred2 logits (fp32) + bias via DVE
            psl = ps_mm.tile([128, TT], F32, tag="mm", name="mml")
            for k in range(FC):
                nc.tensor.matmul(psl[0:NS, :], w2[:, k, :], hT[k],
                                 start=(k == 0), stop=(k == FC - 1))
            lT = P.tile([NS, TT], F32, tag="lT")
            nc.vector.tensor_scalar_add(out=lT, in0=psl[0:NS, :], scalar1=b2)

            # softmax + one-hot, batched across the 4 sub-tiles
            ptl = ps_tp.tile([128, NST, 128], F32, tag="tp", name="tpl")
            for st in range(NST):
                nc.tensor.matmul(ptl[:, st, 0:NS], lT[:, st * 128:(st + 1) * 128],
                                 ident[0:NS, 0:NS], is_transpose=True,
                                 skip_group_check=True)
            ln = P.tile([128, NST, NS], F32, tag="lnat")
            nc.vector.tensor_copy(ln, ptl[:, :, 0:NS])
            mx = P.tile([128, NST], F32, tag="mx")
            nc.vector.tensor_reduce(out=mx, in_=ln, axis=mybir.AxisListType.X,
                                    op=ALU.max)
            oh = P.tile([128, NST, NS], F32, tag="oh")
            nc.vector.tensor_tensor(out=oh, in0=ln,
                                    in1=mx.broadcast_to([128, NST, NS]),
                                    op=ALU.is_equal)
            # exp(l) via tanh: e = (1+u)/(1-u), u = tanh(l/2); logits are
            # O(1) so no max-shift is needed for range safety.
            u = P.tile([128, NST, NS], F32, tag="u")
            nc.scalar.activation(out=u, in_=ln, func=AF.Tanh, scale=0.5)
            num = P.tile([128, NST, NS], F32, tag="num")
            nc.vector.tensor_scalar_add(out=num, in0=u, scalar1=1.0)
            den = P.tile([128, NST, NS], F32, tag="den")
            nc.vector.tensor_scalar(out=den, in0=u, scalar1=-1.0, scalar2=1.0,
                                    op0=ALU.mult, op1=ALU.add)
            nc.vector.reciprocal(out=den, in_=den)
            ex = P.tile([128, NST, NS], F32, tag="ex")
            nc.vector.tensor_mul(out=ex, in0=num, in1=den)
            sm = P.tile([128, NST], F32, tag="sm")
            nc.vector.reduce_sum(out=sm, in_=ex, axis=mybir.AxisListType.X)
            nc.vector.reciprocal(out=sm, in_=sm)
            stt = P.tile([128, NST, NS], F32, tag="stt")
            nc.vector.tensor_tensor(out=stt, in0=ex,
                                    in1=sm.broadcast_to([128, NST, NS]),
                                    op=ALU.mult)
            nc.sync.dma_start(
                out=st_e[t0:t0 + TT, :].rearrange("(a p) n -> p a n", p=128),
                in_=stt)

            # one-hot transposed [NS, TT]
            pto = ps_tp.tile([128, NST, 128], F32, tag="tp", name="tpo")
            for st in range(NST):
                nc.tensor.matmul(pto[0:NS, st, :], oh[:, st, :], ident,
                                 is_transpose=True, skip_group_check=True)
            ohT = P.tile([NS, NST, 128], F16, tag="ohT")
            nc.vector.tensor_copy(ohT, pto[0:NS, :, :])
            ohT2 = ohT.rearrange("p a b -> p (a b)")

            if norm_affine:
                ohT32 = P.tile([NS, NST, 128], F32, tag="ohT32")
                nc.vector.tensor_copy(ohT32, pto[0:NS, :, :])
                for st in range(NST):
                    gsel = ps_aff.tile([128, D], F32, tag="gsel", name="gsel")
                    bsel = ps_aff.tile([128, D], F32, tag="bsel", name="bsel")
                    for hf in range(2):
                        nc.tensor.matmul(gsel[:, hf * F:(hf + 1) * F],
                                         ohT32[:, st, :],
                                         ngam[:, hf * F:(hf + 1) * F],
                                         start=True, stop=True,
                                         skip_group_check=True)
                        nc.tensor.matmul(bsel[:, hf * F:(hf + 1) * F],
                                         ohT32[:, st, :],
                                         nbet[:, hf * F:(hf + 1) * F],
                                         start=True, stop=True,
                                         skip_group_check=True)
                    nc.vector.tensor_mul(out=xn[st], in0=xn[st], in1=gsel)
                    nc.vector.tensor_add(out=xn[st], in0=xn[st], in1=bsel)
                for fc in range(DC):
                    pt = ps_tp.tile([128, NST, 128], F16, tag="tp", name="tp")
                    for st in range(NST):
                        nc.tensor.matmul(pt[:, st, :],
                                         xn[st][:, fc * 128:(fc + 1) * 128],
                                         ident16, is_transpose=True,
                                         skip_group_check=True)
                    xt = P.tile([128, TT], F16, tag="xnT")
                    nc.scalar.activation(out=xt, in_=pt, func=AF.Copy)
                    xnT.append(xt)

            # tr1: y1^T = silu(tr_w1a^T xn^T + emtr^T onehot^T + tb1) (fp16)
            y1T = []
            for mo in range(DC):
                ps = ps_mm.tile([128, TT], F32, tag="mm", name="mmt1")
                for k in range(DC):
                    nc.tensor.matmul(ps, tw1[:, k, mo * 128:(mo + 1) * 128],
                                     xnT[k], start=(k == 0), stop=False)
                nc.tensor.matmul(ps, emtr[:, mo * 128:(mo + 1) * 128], ohT2,
                                 start=False, stop=True)
                y1 = P.tile([128, TT], F16, tag="y1T")
                nc.scalar.activation(out=y1, in_=ps, func=AF.Silu,
                                     bias=tb1[:, mo:mo + 1], scale=1.0)
                y1T.append(y1)

            # tr2 + z-transpose-back interleaved; LN(z) + add xn + store
            zT = []
            zps = {}
            for mo in range(DC):
                ps = ps_mm.tile([128, TT], F32, tag="mm", name="mmt2")
                for k in range(DC):
                    nc.tensor.matmul(ps, tw2[:, k, mo * 128:(mo + 1) * 128],
                                     y1T[k], start=(k == 0), stop=(k == DC - 1))
                z = P.tile([128, TT], F16, tag="zT")
                nc.scalar.activation(out=z, in_=ps, func=AF.Identity,
                                     bias=tb2[:, mo:mo + 1], scale=1.0)
                zT.append(z)
            # transpose back to natural; stats + eviction straight from
            # psum, then the psum bank is released
            mvz = P.tile([128, NST, 2], F32, tag="mvz")
            znat = []
            for st in range(NST):
                zn = P.tile([128, D], F32, tag="znat")
                stats = P.tile([128, 2, 6], F32, tag="bnstz")
                for g in range(2):
                    pt = ps_z.tile([128, 4, 128], F16, tag="ztp", name="ztp")
                    for j in range(4):
                        fc = g * 4 + j
                        nc.tensor.matmul(
                            pt[:, j, :], zT[fc][:, st * 128:(st + 1) * 128],
                            ident16, is_transpose=True, skip_group_check=True)
                    flat = pt.rearrange("p a b -> p (a b)")
                    nc.vector.bn_stats(out=stats[:, g, :], in_=flat)
                    nc.vector.tensor_copy(zn[:, g * 512:(g + 1) * 512], flat)
                nc.vector.bn_aggr(out=mvz[:, st, :], in_=stats)
                znat.append(zn)
            vz = P.tile([128, NST], F32, tag="vz")
            nc.vector.tensor_scalar_add(out=vz, in0=mvz[:, :, 1], scalar1=EPS)
            rz = _rsqrt_newton(nc, P, vz)
            for st in range(NST):
                on = P.tile([128, D], F32, tag="outn")
                nc.vector.tensor_scalar(
                    out=on, in0=znat[st],
                    scalar1=mvz[:, st, 0:1], scalar2=rz[:, st:st + 1],
                    op0=ALU.subtract, op1=ALU.mult)
                if tr_affine:
                    nc.vector.tensor_mul(out=on, in0=on, in1=tgam)
                    nc.vector.tensor_add(out=on, in0=on, in1=tbet)
                nc.vector.tensor_add(out=on, in0=on, in1=xn[st])
                nc.sync.dma_start(out=out_e[t0 + st * 128: t0 + (st + 1) * 128, :],
                                  in_=on)

    return _fix_excess_waits(nc)


_NC_CACHE = {}


def _split16(a):
    hi = a.astype(np.float16)
    lo = (a - hi.astype(np.float32)).astype(np.float16)
    return hi, lo


def kernel(**inputs):
    x = np.ascontiguousarray(np.asarray(inputs["x"], np.float32).reshape(T, D))
    pw1 = np.asarray(inputs["pred_w1"], np.float32)
    pb1 = np.asarray(inputs["pred_b1"], np.float32)
    pw2 = np.asarray(inputs["pred_w2"], np.float32)
    pb2 = np.asarray(inputs["pred_b2"], np.float32)
    ngam = np.asarray(inputs["norm_gamma"], np.float32)
    nbet = np.asarray(inputs["norm_beta"], np.float32)
    emb = np.asarray(inputs["emb"], np.float32)
    tw1 = np.asarray(inputs["tr_w1"], np.float32)
    tb1 = np.asarray(inputs["tr_b1"], np.float32)
    tw2 = np.asarray(inputs["tr_w2"], np.float32)
    tb2 = np.asarray(inputs["tr_b2"], np.float32)
    tgam = np.asarray(inputs["tr_gamma"], np.float32)
    tbet = np.asarray(inputs["tr_beta"], np.float32)

    norm_affine = not (np.all(ngam == 1.0) and np.all(nbet == 0.0))
    tr_affine = not (np.all(tgam == 1.0) and np.all(tbet == 0.0))

    key = (norm_affine, tr_affine)
    if key not in _NC_CACHE:
        _NC_CACHE[key] = build_nc(norm_affine, tr_affine)
    nc = _NC_CACHE[key]

    xt = np.ascontiguousarray(x.T)               # [D, T]
    xth, xtl = _split16(xt)
    w1h, w1l = _split16(pw1)
    emtr = (emb @ tw1[D:]).astype(np.float16)
    shared = {
        "pw1h": w1h, "pw1l": w1l,
        "pb1": np.ascontiguousarray(pb1.reshape(FC, 128).T),
        "pw2": pw2,
        "pb2": pb2.reshape(NS, 1),
        "tw1": np.ascontiguousarray(tw1[:D]).astype(np.float16),
        "tb1": np.ascontiguousarray(tb1.reshape(DC, 128).T),
        "tw2": tw2.astype(np.float16),
        "tb2": np.ascontiguousarray(tb2.reshape(DC, 128).T),
        "emtr": emtr,
    }
    if norm_affine:
        shared["ngam"] = ngam
        shared["nbet"] = nbet
    if tr_affine:
        shared["tgam"] = tgam.reshape(1, D)
        shared["tbet"] = tbet.reshape(1, D)

    in_maps = []
    for c in range(NCORES):
        m = dict(shared, x=x[c * TPC:(c + 1) * TPC],
                 xth=np.ascontiguousarray(xth[:, c * TPC:(c + 1) * TPC]),
                 xtl=np.ascontiguousarray(xtl[:, c * TPC:(c + 1) * TPC]))
        in_maps.append(m)
    kw = {}
    if globals().get("TRACE"):
        kw = dict(trace=True)
    r = run_bass_kernel_spmd(nc, in_maps, list(range(NCORES)), **kw)
    globals()["LAST_RESULTS"] = r
    res = r.results

    out = np.concatenate([res[c]["out"] for c in range(NCORES)], axis=0)
    states = np.concatenate([res[c]["states"] for c in range(NCORES)], axis=0)
    return out.reshape(B, S, D), states.reshape(B, S, NS)


# revision 28
# speedup vs baseline: 1.0353x; 1.0353x over previous
"""Trainium2 Bass kernel for nn_EpigeneticNormalization.

Math (per token, D=1024, F=512, NS=3):
  h      = silu(x @ pred_w1 + pred_b1)
  logits = h @ pred_w2 + pred_b2
  states = softmax(logits);  sid = argmax(logits)
  xn     = LN(x) * norm_gamma[sid] + norm_beta[sid]
  t      = concat(xn, emb[sid])
  y      = silu(t @ tr_w1 + tr_b1) @ tr_w2 + tr_b2
  out    = xn + LN(y) * tr_gamma + tr_beta,  states

Device strategy (8 cores, token-parallel, 4096 tokens/core):
  - Activations in "transposed" layout [feature, token] for matmuls
    (contract dim on partitions), natural [token, feature] for LN.
  - x^T is host-pretransposed and fp16-hi/lo split; pred1 runs as a
    3-pass fp16 matmul (hi*hi + hi*lo + lo*hi) which is bit-accurate
    enough to keep argmax stable (min top-2 logit gap ~5e-6; measured
    fp16x3 logit error <1e-6).  pred2 stays fp32.
  - tr path in fp16 (full PE rate).  emb[sid] @ tr_w1[D:] is folded on
    host into a [3,1024] matrix; the gather becomes a K=3 one-hot
    matmul accumulated into the tr1 psum.
  - ACT uses only the silu table set (silu/tanh/identity/copy):
    softmax's exp goes through tanh, LN's rsqrt is a DVE Newton
    iteration on a bitcast seed.  Zero ACT table switches in steady
    state.
  - PE transposes (xn^T, z back to natural) are interleaved between
    matmul groups to keep HAM from re-throttling the PE clock.
"""
import json
import sys

sys.path.insert(0, "/opt/trn_rl_repo")

import numpy as np

import concourse.bass as bass
import concourse.tile as tile
from concourse import mybir
from concourse.bass_utils import run_bass_kernel_spmd
from concourse.masks import make_identity

F32 = mybir.dt.float32
F16 = mybir.dt.float16
I32 = mybir.dt.int32
AF = mybir.ActivationFunctionType
ALU = mybir.AluOpType

NCORES = 8
B, S, D, NS = 4, 8192, 1024, 3
F = D // 2
T = B * S                 # 32768 tokens total
TPC = T // NCORES         # 4096 tokens per core
TT = 512                  # tokens per macro tile
NMACRO = TPC // TT        # 8
NST = TT // 128           # 4 sub-tiles of 128 tokens
DC = D // 128             # 8 feature chunks
FC = F // 128             # 4 pred-hidden chunks
EPS = 1e-5
MAGIC = 0x5F3759DF

_WAIT_LIMIT = 1


def _fix_excess_waits(nc, limit=_WAIT_LIMIT):
    """This walrus build rejects >1 sync-wait per instruction; split the
    excess onto same-engine Drain instructions inserted just before."""
    raw = nc.to_json_bytes()
    d = json.loads(raw)
    n = [0]

    def mk(engine, waits, debug):
        n[0] += 1
        return {"opcode": "Drain", "engine": engine, "ins": [], "outs": [],
                "is_reset_sema": False, "name": f"I-waitfix-{n[0]}",
                "sync_info": {"on_update": [], "on_wait": waits},
                "debug": debug}

    for f in d["functions"]:
        for b in f["blocks"]:
            out = []
            for inst in b["instructions"]:
                si = inst.get("sync_info") or {}
                waits = si.get("on_wait") or []
                if len(waits) > limit:
                    excess = waits[: len(waits) - limit]
                    si["on_wait"] = waits[len(waits) - limit:]
                    inst["sync_info"] = si
                    for i in range(0, len(excess), limit):
                        out.append(mk(inst["engine"], excess[i:i + limit],
                                      inst.get("debug")))
                out.append(inst)
            b["instructions"] = out
    blob = json.dumps(d).encode()
    nc.to_json_bytes = lambda: blob
    return nc


def _rsqrt_newton(nc, P, x, n_iter=3):
    """out = 1/sqrt(x) elementwise on DVE (no ACT table use).

    x: [128, W] fp32 sbuf AP.  Returns a [128, W] fp32 tile.
    """
    W = x.shape[1]
    xh = P.tile([128, W], F32, tag="nwxh")
    nc.vector.tensor_scalar_mul(out=xh, in0=x, scalar1=0.5)
    ibits = P.tile([128, W], I32, tag="nwi")
    nc.vector.tensor_scalar(out=ibits, in0=x.bitcast(I32), scalar1=1,
                            scalar2=None, op0=ALU.arith_shift_right)
    y = P.tile([128, W], F32, tag="nwy")
    nc.vector.tensor_scalar(out=y.bitcast(I32), in0=ibits, scalar1=-1,
                            scalar2=MAGIC, op0=ALU.mult, op1=ALU.add)
    t1 = P.tile([128, W], F32, tag="nwt1")
    t2 = P.tile([128, W], F32, tag="nwt2")
    for _ in range(n_iter):
        nc.vector.tensor_mul(out=t1, in0=y, in1=y)
        nc.vector.tensor_mul(out=t2, in0=t1, in1=xh)
        nc.vector.tensor_scalar(out=t2, in0=t2, scalar1=-1.0, scalar2=1.5,
                                op0=ALU.mult, op1=ALU.add)
        nc.vector.tensor_mul(out=y, in0=y, in1=t2)
    return y


def build_nc(norm_affine: bool, tr_affine: bool):
    nc = bass.Bass()

    x_e = nc.declare_dram_parameter("x", [TPC, D], F32, isOutput=False)
    xth_e = nc.declare_dram_parameter("xth", [D, TPC], F16, isOutput=False)
    xtl_e = nc.declare_dram_parameter("xtl", [D, TPC], F16, isOutput=False)
    w1h_e = nc.declare_dram_parameter("pw1h", [D, F], F16, isOutput=False)
    w1l_e = nc.declare_dram_parameter("pw1l", [D, F], F16, isOutput=False)
    b1_e = nc.declare_dram_parameter("pb1", [128, FC], F32, isOutput=False)
    w2_e = nc.declare_dram_parameter("pw2", [F, NS], F32, isOutput=False)
    b2_e = nc.declare_dram_parameter("pb2", [NS, 1], F32, isOutput=False)
    tw1_e = nc.declare_dram_parameter("tw1", [D, D], F16, isOutput=False)
    tb1_e = nc.declare_dram_parameter("tb1", [128, DC], F32, isOutput=False)
    tw2_e = nc.declare_dram_parameter("tw2", [D, D], F16, isOutput=False)
    tb2_e = nc.declare_dram_parameter("tb2", [128, DC], F32, isOutput=False)
    emtr_e = nc.declare_dram_parameter("emtr", [NS, D], F16, isOutput=False)
    if norm_affine:
        ng_e = nc.declare_dram_parameter("ngam", [NS, D], F32, isOutput=False)
        nb_e = nc.declare_dram_parameter("nbet", [NS, D], F32, isOutput=False)
    if tr_affine:
        tg_e = nc.declare_dram_parameter("tgam", [1, D], F32, isOutput=False)
        tb_e = nc.declare_dram_parameter("tbet", [1, D], F32, isOutput=False)
    out_e = nc.declare_dram_parameter("out", [TPC, D], F32, isOutput=True)
    st_e = nc.declare_dram_parameter("states", [TPC, NS], F32, isOutput=True)

    from contextlib import ExitStack
    BUFS = {"xnat": 4, "xn": 10, "hT": 6, "xth": 10, "xtl": 10,
            "xnT": 10, "y1T": 8, "zT": 8, "znat": 4, "outn": 2, "lT": 2,
            "ohT": 3, "ohT32": 2, "tiny": 12}
    with tile.TileContext(nc) as tc, ExitStack() as ctx:
        pools = {t: ctx.enter_context(tc.tile_pool(name=t, bufs=n))
                 for t, n in BUFS.items()}

        class _P:
            def tile(self, shape, dt, tag=None, name=None):
                return pools[tag if tag in pools else "tiny"].tile(
                    shape, dt, tag=tag, name=name or tag)
        P = _P()
        cons = ctx.enter_context(tc.tile_pool(name="cons", bufs=1))
        ps_tp = ctx.enter_context(tc.tile_pool(
            name="ps_tp", bufs=2, space="PSUM"))
        ps_z = ctx.enter_context(tc.tile_pool(
            name="ps_z", bufs=2, space="PSUM"))
        ps_mm = ctx.enter_context(tc.tile_pool(
            name="ps_mm", bufs=3 if norm_affine else 4, space="PSUM"))
        if norm_affine:
            ps_aff = ctx.enter_context(tc.tile_pool(
                name="ps_aff", bufs=1, space="PSUM"))

        # ---- constants / weights (loaded once) ----
        ident = cons.tile([128, 128], F32)
        make_identity(nc, ident)
        ident16 = cons.tile([128, 128], F16)
        make_identity(nc, ident16)

        w1h = cons.tile([128, DC, F], F16)
        w1l = cons.tile([128, DC, F], F16)
        for c in range(DC):
            nc.gpsimd.dma_start(out=w1h[:, c, :], in_=w1h_e[c * 128:(c + 1) * 128, :])
            nc.gpsimd.dma_start(out=w1l[:, c, :], in_=w1l_e[c * 128:(c + 1) * 128, :])
        w2 = cons.tile([128, FC, NS], F32)
        for c in range(FC):
            nc.gpsimd.dma_start(out=w2[:, c, :], in_=w2_e[c * 128:(c + 1) * 128, :])
        tw1 = cons.tile([128, DC, D], F16)
        tw2 = cons.tile([128, DC, D], F16)
        for c in range(DC):
            nc.gpsimd.dma_start(out=tw1[:, c, :], in_=tw1_e[c * 128:(c + 1) * 128, :])
            nc.gpsimd.dma_start(out=tw2[:, c, :], in_=tw2_e[c * 128:(c + 1) * 128, :])
        emtr = cons.tile([NS, D], F16)
        nc.gpsimd.dma_start(out=emtr, in_=emtr_e[:])
        b1 = cons.tile([128, FC], F32)
        nc.gpsimd.dma_start(out=b1, in_=b1_e[:])
        b2 = cons.tile([NS, 1], F32)
        nc.gpsimd.dma_start(out=b2, in_=b2_e[:])
        tb1 = cons.tile([128, DC], F32)
        nc.gpsimd.dma_start(out=tb1, in_=tb1_e[:])
        tb2 = cons.tile([128, DC], F32)
        nc.gpsimd.dma_start(out=tb2, in_=tb2_e[:])
        if norm_affine:
            ngam = cons.tile([NS, D], F32)
            nc.gpsimd.dma_start(out=ngam, in_=ng_e[:])
            nbet = cons.tile([NS, D], F32)
            nc.gpsimd.dma_start(out=nbet, in_=nb_e[:])
        if tr_affine:
            tgam = cons.tile([128, D], F32)
            tbet = cons.tile([128, D], F32)
            bg = tg_e[:]
            bb = tb_e[:]
            nc.gpsimd.dma_start(out=tgam, in_=bass.AP(
                tensor=bg.tensor, offset=bg.offset, ap=[[0, 128]] + bg.ap[1:]))
            nc.gpsimd.dma_start(out=tbet, in_=bass.AP(
                tensor=bb.tensor, offset=bb.offset, ap=[[0, 128]] + bb.ap[1:]))

        # ---- main loop over macro tiles (LN-x stage pipelined 1 ahead) ----
        def stage_a(mi):
            """DMA x/x^T for macro mi, compute LN(x) stats + xn."""
            t0 = mi * TT
            xth, xtl = [], []
            for fc in range(DC):
                th = P.tile([128, TT], F16, tag="xth")
                nc.sync.dma_start(out=th,
                                  in_=xth_e[fc * 128:(fc + 1) * 128, t0:t0 + TT])
                xth.append(th)
                tl = P.tile([128, TT], F16, tag="xtl")
                nc.sync.dma_start(out=tl,
                                  in_=xtl_e[fc * 128:(fc + 1) * 128, t0:t0 + TT])
                xtl.append(tl)
            xnat = []
            for st in range(NST):
                xt = P.tile([128, D], F32, tag="xnat")
                nc.sync.dma_start(out=xt, in_=x_e[t0 + st * 128: t0 + (st + 1) * 128, :])
                xnat.append(xt)
            mvall = P.tile([128, NST, 2], F32, tag="mvall")
            for st in range(NST):
                stats = P.tile([128, 2, 6], F32, tag="bnst")
                xr = xnat[st].rearrange("p (a b) -> p a b", a=2)
                nc.vector.bn_stats(out=stats[:, 0, :], in_=xr[:, 0, :])
                nc.vector.bn_stats(out=stats[:, 1, :], in_=xr[:, 1, :])
                nc.vector.bn_aggr(out=mvall[:, st, :], in_=stats)
            vv = P.tile([128, NST], F32, tag="vv")
            nc.vector.tensor_scalar_add(out=vv, in0=mvall[:, :, 1], scalar1=EPS)
            rr = _rsqrt_newton(nc, P, vv)
            xn = []
            for st in range(NST):
                xnt = P.tile([128, D], F16, tag="xn")
                nc.vector.tensor_scalar(out=xnt, in0=xnat[st],
                                        scalar1=mvall[:, st, 0:1],
                                        scalar2=rr[:, st:st + 1],
                                        op0=ALU.subtract, op1=ALU.mult)
                xn.append(xnt)
            return xth, xtl, xn

        staged = stage_a(0)
        for mi in range(NMACRO):
            t0 = mi * TT
            xth, xtl, xn = staged
            if mi + 1 < NMACRO:
                staged = stage_a(mi + 1)

            # pred1 (fp16 x3 passes)
            hT = []
            for mo in range(FC):
                ps = ps_mm.tile([128, TT], F32, tag="mm", name="mm")
                for k in range(DC):
                    nc.tensor.matmul(ps, w1h[:, k, mo * 128:(mo + 1) * 128],
                                     xth[k], start=(k == 0), stop=False)
                for k in range(DC):
                    nc.tensor.matmul(ps, w1h[:, k, mo * 128:(mo + 1) * 128],
                                     xtl[k], start=False, stop=False)
                for k in range(DC):
                    nc.tensor.matmul(ps, w1l[:, k, mo * 128:(mo + 1) * 128],
                                     xth[k], start=False,
                                     stop=(k == DC - 1))
                h = P.tile([128, TT], F32, tag="hT")
                nc.scalar.activation(out=h, in_=ps, func=AF.Silu,
                                     bias=b1[:, mo:mo + 1], scale=1.0)
                hT.append(h)

            # xn^T (fp16 PE transposes)
            xnT = []
            if not norm_affine:
                for fc in range(DC):
                    pt = ps_tp.tile([128, NST, 128], F16, tag="tp", name="tp")
                    for st in range(NST):
                        nc.tensor.matmul(pt[:, st, :],
                                         xn[st][:, fc * 128:(fc + 1) * 128],
                                         ident16, is_transpose=True,
                                         skip_group_check=True)
                    xt = P.tile([128, TT], F16, tag="xnT")
                    nc.scalar.activation(out=xt, in_=pt, func=AF.Copy)
                    xnT.append(xt)

            # pred2 logits (fp32) + bias via DVE
            psl = ps_mm.tile([128, TT], F32, tag="mm", name="mml")
            for k in range(FC):
                nc.tensor.matmul(psl[0:NS, :], w2[:, k, :], hT[k],
                                 start=(k == 0), stop=(k == FC - 1))
            lT = P.tile([NS, TT], F32, tag="lT")
            nc.vector.tensor_scalar_add(out=lT, in0=psl[0:NS, :], scalar1=b2)

            # softmax + one-hot, batched across the 4 sub-tiles
            ptl = ps_tp.tile([128, NST, 128], F32, tag="tp", name="tpl")
            for st in range(NST):
                nc.tensor.matmul(ptl[:, st, 0:NS], lT[:, st * 128:(st + 1) * 128],
                                 ident[0:NS, 0:NS], is_transpose=True,
                                 skip_group_check=True)
            ln = P.tile([128, NST, NS], F32, tag="lnat")
            nc.vector.tensor_copy(ln, ptl[:, :, 0:NS])
            mx = P.tile([128, NST], F32, tag="mx")
            nc.vector.tensor_reduce(out=mx, in_=ln, axis=mybir.AxisListType.X,
                                    op=ALU.max)
            oh = P.tile([128, NST, NS], F32, tag="oh")
            nc.vector.tensor_tensor(out=oh, in0=ln,
                                    in1=mx.broadcast_to([128, NST, NS]),
                                    op=ALU.is_equal)
            # exp(l) via tanh: e = (1+u)/(1-u), u = tanh(l/2); logits are
            # O(1) so no max-shift is needed for range safety.
            u = P.tile([128, NST, NS], F32, tag="u")
            nc.scalar.activation(out=u, in_=ln, func=AF.Tanh, scale=0.5)
            num = P.tile([128, NST, NS], F32, tag="num")
            nc.vector.tensor_scalar_add(out=num, in0=u, scalar1=1.0)
            den = P.tile([128, NST, NS], F32, tag="den")
            nc.vector.tensor_scalar(out=den, in0=u, scalar1=-1.0, scalar2=1.0,
                                    op0=ALU.mult, op1=ALU.add)
            nc.vector.reciprocal(out=den, in_=den)
            ex = P.tile([128, NST, NS], F32, tag="ex")
            nc.vector.tensor_mul(out=ex, in0=num, in1=den)
            sm = P.tile([128, NST], F32, tag="sm")
            nc.vector.reduce_sum(out=sm, in_=ex, axis=mybir.AxisListType.X)
            nc.vector.reciprocal(out=sm, in_=sm)
            stt = P.tile([128, NST, NS], F32, tag="stt")
            nc.vector.tensor_tensor(out=stt, in0=ex,
                                    in1=sm.broadcast_to([128, NST, NS]),
                                    op=ALU.mult)
            nc.sync.dma_start(
                out=st_e[t0:t0 + TT, :].rearrange("(a p) n -> p a n", p=128),
                in_=stt)

            # one-hot transposed [NS, TT]
            pto = ps_tp.tile([128, NST, 128], F32, tag="tp", name="tpo")
            for st in range(NST):
                nc.tensor.matmul(pto[0:NS, st, :], oh[:, st, :], ident,
                                 is_transpose=True, skip_group_check=True)
            ohT = P.tile([NS, NST, 128], F16, tag="ohT")
            nc.vector.tensor_copy(ohT, pto[0:NS, :, :])
            ohT2 = ohT.rearrange("p a b -> p (a b)")

            if norm_affine:
                ohT32 = P.tile([NS, NST, 128], F32, tag="ohT32")
                nc.vector.tensor_copy(ohT32, pto[0:NS, :, :])
                for st in range(NST):
                    gsel = ps_aff.tile([128, D], F32, tag="gsel", name="gsel")
                    bsel = ps_aff.tile([128, D], F32, tag="bsel", name="bsel")
                    for hf in range(2):
                        nc.tensor.matmul(gsel[:, hf * F:(hf + 1) * F],
                                         ohT32[:, st, :],
                                         ngam[:, hf * F:(hf + 1) * F],
                                         start=True, stop=True,
                                         skip_group_check=True)
                        nc.tensor.matmul(bsel[:, hf * F:(hf + 1) * F],
                                         ohT32[:, st, :],
                                         nbet[:, hf * F:(hf + 1) * F],
                                         start=True, stop=True,
                                         skip_group_check=True)
                    nc.vector.tensor_mul(out=xn[st], in0=xn[st], in1=gsel)
                    nc.vector.tensor_add(out=xn[st], in0=xn[st], in1=bsel)
                xnT = []
                for fc in range(DC):
                    pt = ps_tp.tile([128, NST, 128], F16, tag="tp", name="tp")
                    for st in range(NST):
                        nc.tensor.matmul(pt[:, st, :],
                                         xn[st][:, fc * 128:(fc + 1) * 128],
                                         ident16, is_transpose=True,
                                         skip_group_check=True)
                    xt = P.tile([128, TT], F16, tag="xnT")
                    nc.scalar.activation(out=xt, in_=pt, func=AF.Copy)
                    xnT.append(xt)

            # tr1: y1^T = silu(tr_w1a^T xn^T + emtr^T onehot^T + tb1) (fp16)
            y1T = []
            for mo in range(DC):
                ps = ps_mm.tile([128, TT], F32, tag="mm", name="mmt1")
                for k in range(DC):
                    nc.tensor.matmul(ps, tw1[:, k, mo * 128:(mo + 1) * 128],
                                     xnT[k], start=(k == 0), stop=False)
                nc.tensor.matmul(ps, emtr[:, mo * 128:(mo + 1) * 128], ohT2,
                                 start=False, stop=True)
                y1 = P.tile([128, TT], F16, tag="y1T")
                nc.scalar.activation(out=y1, in_=ps, func=AF.Silu,
                                     bias=tb1[:, mo:mo + 1], scale=1.0)
                y1T.append(y1)

            # tr2 + z-transpose-back interleaved; LN(z) + add xn + store
            zT = []
            zps = {}
            for mo in range(DC):
                ps = ps_mm.tile([128, TT], F32, tag="mm", name="mmt2")
                for k in range(DC):
                    nc.tensor.matmul(ps, tw2[:, k, mo * 128:(mo + 1) * 128],
                                     y1T[k], start=(k == 0), stop=(k == DC - 1))
                z = P.tile([128, TT], F16, tag="zT")
                nc.scalar.activation(out=z, in_=ps, func=AF.Identity,
                                     bias=tb2[:, mo:mo + 1], scale=1.0)
                zT.append(z)
            # transpose back to natural; stats + eviction straight from
            # psum, then the psum bank is released
            mvz = P.tile([128, NST, 2], F32, tag="mvz")
            znat = []
            for st in range(NST):
                zn = P.tile([128, D], F32, tag="znat")
                stats = P.tile([128, 2, 6], F32, tag="bnstz")
                for g in range(2):
                    pt = ps_z.tile([128, 4, 128], F16, tag="ztp", name="ztp")
                    for j in range(4):
                        fc = g * 4 + j
                        nc.tensor.matmul(
                            pt[:, j, :], zT[fc][:, st * 128:(st + 1) * 128],
                            ident16, is_transpose=True, skip_group_check=True)
                    flat = pt.rearrange("p a b -> p (a b)")
                    nc.vector.bn_stats(out=stats[:, g, :], in_=flat)
                    nc.vector.tensor_copy(zn[:, g * 512:(g + 1) * 512], flat)
                nc.vector.bn_aggr(out=mvz[:, st, :], in_=stats)
                znat.append(zn)
            vz = P.tile([128, NST], F32, tag="vz")
            nc.vector.tensor_scalar_add(out=vz, in0=mvz[:, :, 1], scalar1=EPS)
            rz = _rsqrt_newton(nc, P, vz)
            for st in range(NST):
                on = P.tile([128, D], F32, tag="outn")
                nc.vector.tensor_scalar(
                    out=on, in0=znat[st],
                    scalar1=mvz[:, st, 0:1], scalar2=rz[:, st:st + 1],
                    op0=ALU.subtract, op1=ALU.mult)
                if tr_affine:
                    nc.vector.tensor_mul(out=on, in0=on, in1=tgam)
                    nc.vector.tensor_add(out=on, in0=on, in1=tbet)
                nc.vector.tensor_add(out=on, in0=on, in1=xn[st])
                nc.sync.dma_start(out=out_e[t0 + st * 128: t0 + (st + 1) * 128, :],
                                  in_=on)

    return _fix_excess_waits(nc)


_NC_CACHE = {}


def _split16(a):
    hi = a.astype(np.float16)
    lo = (a - hi.astype(np.float32)).astype(np.float16)
    return hi, lo


def kernel(**inputs):
    x = np.ascontiguousarray(np.asarray(inputs["x"], np.float32).reshape(T, D))
    pw1 = np.asarray(inputs["pred_w1"], np.float32)
    pb1 = np.asarray(inputs["pred_b1"], np.float32)
    pw2 = np.asarray(inputs["pred_w2"], np.float32)
    pb2 = np.asarray(inputs["pred_b2"], np.float32)
    ngam = np.asarray(inputs["norm_gamma"], np.float32)
    nbet = np.asarray(inputs["norm_beta"], np.float32)
    emb = np.asarray(inputs["emb"], np.float32)
    tw1 = np.asarray(inputs["tr_w1"], np.float32)
    tb1 = np.asarray(inputs["tr_b1"], np.float32)
    tw2 = np.asarray(inputs["tr_w2"], np.float32)
    tb2 = np.asarray(inputs["tr_b2"], np.float32)
    tgam = np.asarray(inputs["tr_gamma"], np.float32)
    tbet = np.asarray(inputs["tr_beta"], np.float32)

    norm_affine = not (np.all(ngam == 1.0) and np.all(nbet == 0.0))
    tr_affine = not (np.all(tgam == 1.0) and np.all(tbet == 0.0))

    key = (norm_affine, tr_affine)
    if key not in _NC_CACHE:
        _NC_CACHE[key] = build_nc(norm_affine, tr_affine)
    nc = _NC_CACHE[key]

    xt = np.ascontiguousarray(x.T)               # [D, T]
    xth, xtl = _split16(xt)
    w1h, w1l = _split16(pw1)
    emtr = (emb @ tw1[D:]).astype(np.float16)
    shared = {
        "pw1h": w1h, "pw1l": w1l,
        "pb1": np.ascontiguousarray(pb1.reshape(FC, 128).T),
        "pw2": pw2,
        "pb2": pb2.reshape(NS, 1),
        "tw1": np.ascontiguousarray(tw1[:D]).astype(np.float16),
        "tb1": np.ascontiguousarray(tb1.reshape(DC, 128).T),
        "tw2": tw2.astype(np.float16),
        "tb2": np.ascontiguousarray(tb2.reshape(DC, 128).T),
        "emtr": emtr,
    }
    if norm_affine:
        shared["ngam"] = ngam
        shared["nbet"] = nbet
    if tr_affine:
        shared["tgam"] = tgam.reshape(1, D)
        shared["tbet"] = tbet.reshape(1, D)

    in_maps = []
    for c in range(NCORES):
        m = dict(shared, x=x[c * TPC:(c + 1) * TPC],
                 xth=np.ascontiguousarray(xth[:, c * TPC:(c + 1) * TPC]),
                 xtl=np.ascontiguousarray(xtl[:, c * TPC:(c + 1) * TPC]))
        in_maps.append(m)
    kw = {}
    if globals().get("TRACE"):
        kw = dict(trace=True)
    r = run_bass_kernel_spmd(nc, in_maps, list(range(NCORES)), **kw)
    globals()["LAST_RESULTS"] = r
    res = r.results

    out = np.concatenate([res[c]["out"] for c in range(NCORES)], axis=0)
    states = np.concatenate([res[c]["states"] for c in range(NCORES)], axis=0)
    return out.reshape(B, S, D), states.reshape(B, S, NS)
